# revision 58
# baseline (speedup 1.0000x reference)
"""Single-head causal attention on 8 Trainium2 NeuronCores.

Problem: x[B=8, T=2048, E=1024] fp32, Wq/Wk/Wv [E, H=64] fp32.
    q = x @ Wq; k = x @ Wk; v = x @ Wv
    out = softmax(causal(q @ k^T / sqrt(H))) @ v          -> [8, 2048, 64]

Sharding: pure data parallel, one batch element per core; weights replicated.

Per-core kernel design (transposed-scores formulation):
  - x loads as one descriptor-efficient DMA per 512-row group ([p, i, e]
    layout, 4KB descriptors); group 0 is split into e-quarters so the
    first PE transposes start ~2us in.  Weights are host-prepacked into a
    single contiguous [Wv|Wk|Wq] tensor (one DMA, split in two halves).
  - xT[e, t] built from x via PE transpose-mode (fp32, 2 cyc/row - the
    BIR verifier requires f32r matmul operands to come from rounding
    instructions, so DMA-fed transposes cannot run at the f32r rate);
    the psum->sbuf copies (DVE, ACT in the prologue) do the f32r rounding.
  - Projections: [Wv|Wk] packed -> one M=128 matmul chain gives vT on psum
    rows 0:64 and kT on rows 64:128; [0|Wq] (zero block built on-chip)
    gives qT on rows 64:128.  kT and qT land on the SAME partitions, so
    score matmuls need no partition shift at all (operands at base 64).
  - vT -> small PE transposes -> vaug[:, j, :] = [v | 1] tiles [128, 65];
    the ones column makes the softmax denominator fall out of the AV
    matmul for free.
  - scoresT[s, t] = kT_j.T @ qT into PSUM, diagonal blocks narrowed to
    the live column range; exp(scale*x) on ACT straight from psum (no
    pre-mask); the causal triangle is zeroed POST-exp on the SBUF tile by
    a Pool-engine affine_select, off the score->exp critical path.
    exp without max-subtraction is safe: |scores| <~ 6.
  - outT[65, 512] accumulates vaug_j.T @ expT_j over j (lag-3 software
    pipeline vs exp); row 64 = softmax denominator.  Small PE transposes
    back to [t, 65], rows scaled by the reciprocal denominator via ACT
    activation(Copy, scale=rcp), one store DMA per group (the last group
    staggers per-chunk copies/stores across DVE/ACT + both HWDGE queues
    to shorten the serial tail).
  - Heavy matmuls run as float32r (1 cyc/row at N>=256, ~tf32-grade:
    measured rel err 2.6e-4 end to end) with an fp32 fallback
    (ATTN_MM_DTYPE=f32, 4 cycles/row).  Engine/ISA legality learned the
    hard way: GPSIMD cannot touch PSUM, f32r transpose-mode and
    tile_position column offsets fail the walrus ISA check, bf16/f32r
    operand mixes are rejected (NCC_IBIR034).
  - Software pipeline: window g round-robins attention(g) with group
    g-1's normalize/store and a filler stream of loads/transposes/
    projections(g+1) chained into attention(g+1) pre-emission; group 0's
    projection matmuls interleave per-chunk with its transposes.
"""

import os

import numpy as np

import concourse.bacc as bacc
import concourse.bass as bass
import concourse.tile as tile
from concourse import mybir
from concourse.masks import make_identity

B, T, E, H = 8, 2048, 1024, 64
P = 128                      # SBUF partitions
NE = E // P                  # 8 e-chunks
NT = T // P                  # 16 t-chunks (also s-chunks)
GW = 512                     # t-group width (matmul moving-operand max, fp32)
NG = T // GW                 # 4 t-groups
CPG = GW // P                # 4 chunks per group
F32 = mybir.dt.float32

# Matmul dtype for the heavy matmuls: "f32r" (fast) or "f32" (exact).
MM_DTYPE = os.environ.get("ATTN_MM_DTYPE", "f32r")

_NC_CACHE: dict = {}




def build_attention_nc(mm_dtype: str = "f32r", repeat: int = 1) -> bass.Bass:
    """Build the single-core Bass program (SPMD across cores via in_maps)."""
    mm_dt = {"f32": F32, "f32r": mybir.dt.float32r, "bf16": F32}[mm_dtype]
    # Transposes stay fp32 (2 cyc/row): the BIR verifier requires every
    # operand of an f32r matmul to be produced by an explicitly-rounding
    # instruction, and DMA-filled x tiles are not; a bf16 identity trips
    # NCC_IBIR034 (no 32-bit/non-32-bit mixing).  The psum->sbuf copies
    # after each transpose are the rounding step that feeds f32r matmuls.
    id_dt = F32
    tr_dt = F32

    nc = bacc.Bacc("TRN2", target_bir_lowering=False, debug=False)
    x_d = nc.dram_tensor("x", [T, E], F32, kind="ExternalInput").ap()
    # host-prepacked [Wv | Wk | 0 | Wq]: one wide contiguous weight tensor
    # loads in a single DMA; the zero block pads Wq to a full M=128
    # stationary so qT lands on psum rows 64:128 without tile_position
    # (f32r matmuls fail the walrus ISA check at column offsets).
    w_d = nc.dram_tensor("Wvkq", [E, 3 * H], F32, kind="ExternalInput").ap()
    out_d = nc.dram_tensor("out", [T, H], F32, kind="ExternalOutput").ap()

    with tile.TileContext(nc) as tc:
        with (
            tc.tile_pool(name="const", bufs=1) as const,
            tc.tile_pool(name="xin", bufs=2) as xin,
            tc.tile_pool(name="xt", bufs=2) as xtp,
            tc.tile_pool(name="proj", bufs=1) as projp,
            tc.tile_pool(name="vaug", bufs=1) as vaugp,
            tc.tile_pool(name="expt", bufs=8) as exptp,
            tc.tile_pool(name="outs", bufs=4) as outsp,
            tc.tile_pool(name="ps_sc", bufs=2, space="PSUM") as ps_sc_p,
            tc.tile_pool(name="ps_tr", bufs=2, space="PSUM") as ps_tr_p,
            tc.tile_pool(name="ps_pm", bufs=2, space="PSUM") as ps_pm_p,
            tc.tile_pool(name="ps_av", bufs=2, space="PSUM") as ps_av_p,
        ):
            # --- constants ---------------------------------------------------
            identf = const.tile([P, P], F32)
            make_identity(nc, identf)
            ident = const.tile([P, P], id_dt, tag="idc")
            nc.vector.tensor_copy(ident, identf)
            # weights, e-major: [p, c, h] with e = c*128 + p.  Wv and Wk are
            # packed so one M=128 matmul yields vT on psum rows 0:64 and kT
            # on rows 64:128; Wq runs separately with its psum output placed
            # at base partition 64 (PE tile_position col=64), so kT and qT
            # land on the SAME partitions and scores need no partition shift.
            w_f = const.tile([P, NE, 3 * H], F32, tag="wf")
            wsrc = w_d.rearrange("(c p) h -> p c h", p=P)
            nc.scalar.dma_start(out=w_f[:, :NE // 2], in_=wsrc[:, :NE // 2])
            nc.scalar.dma_start(out=w_f[:, NE // 2:], in_=wsrc[:, NE // 2:])
            w_c = const.tile([P, NE, 4 * H], mm_dt, tag="wc")
            nc.gpsimd.memset(w_c[:, :, 2 * H:3 * H].bitcast(F32), 0.0)
            for c0 in range(0, NE, 2):
                pair = slice(c0, c0 + 2)
                nc.scalar.copy(w_c[:, pair, :2 * H], w_f[:, pair, :2 * H])
                nc.scalar.copy(w_c[:, pair, 3 * H:], w_f[:, pair, 2 * H:])
            wvk = w_c[:, :, :2 * H]
            wq = w_c[:, :, 2 * H:]
            ones = const.tile([P, NT, 1], F32, tag="ones")
            nc.gpsimd.memset(ones, 1.0)
            # dummy exp so the ACT table load (~1.3us) happens during the
            # DMA-bound fill, not on attention(0)'s critical path
            scr = const.tile([P, 1], F32, tag="scr")
            nc.scalar.activation(scr, ones[:, 0, :],
                                 mybir.ActivationFunctionType.Exp)

            # persistent per-iteration state (allocated fresh each repeat)
            # Variable group widths: a small first group starts attention
            # ~6us earlier during the DMA-bound fill, and a small last
            # group halves the ACT-bound final window.
            GROUPS = [(0, 512), (512, 512), (1024, 512), (1536, 512)]

            def body(_iv=None, staged=False):
                # qT/kT live on partitions 64:128 only; vT on 0:64.
                qT = projp.tile([P, T], mm_dt, tag="qt")
                kT = projp.tile([P, T], mm_dt, tag="kt")
                vT = projp.tile([H, T], F32, tag="vt")
                # vaug[s, j, :] = [v | 1] per s-chunk j; ones column via DVE
                # copy (f32r memset fails the walrus ISA check)
                vaug = vaugp.tile([P, NT, H + 1], mm_dt, tag="vaug")
                nc.vector.tensor_copy(vaug[:, :, H:H + 1], ones)

                # one-time absorber: PE picks up the Pool-engine sem for the
                # identity constant ahead of the first transposes
                dmy = ps_tr_p.tile([1, P], id_dt, tag="tr", name="dmy0")
                nc.tensor.transpose(dmy, ident[:, :1], ident)

                def loads(gi):
                    # one tile [p, i, e] holding the group rows (t = g0 +
                    # i*128 + p); group 0 is split into e-quarters so the
                    # first transposes can start ~1.5us in, later groups
                    # use halves.
                    g0, W = GROUPS[gi]
                    cpg = W // P
                    xt_in = xin.tile([P, cpg, E], F32, tag="xin", name="xin")
                    src = x_d[g0:g0 + W, :]
                    nq = 4 if gi == 0 else 2
                    for q in range(nq):
                        lo, hi = q * E // nq, (q + 1) * E // nq
                        nc.sync.dma_start(
                            out=xt_in[:, :, lo:hi],
                            in_=src[:, lo:hi].rearrange(
                                "(i p) e -> p i e", p=P))
                    return xt_in

                def tp_units(gi, xt_in, prologue=False):
                    """x-transpose + q/k/v projections + vaug build for group
                    gi (pipeline filler units).  In the prologue (group 0, no
                    attention to overlap) the projection matmuls interleave
                    per-chunk with the transposes so only the last chunk's
                    work remains after the final e-quarter lands."""
                    g0, W = GROUPS[gi]
                    cpg = W // P
                    jb0 = g0 // P
                    xT = xtp.tile([P, NE, W], mm_dt, tag="xt", name="xT")
                    ps_vk = ps_pm_p.tile([P, GW], F32, tag="pm", name="psvk")
                    ps_q = ps_pm_p.tile([P, GW], F32, tag="pm", name="psq")

                    def emit_trb(c):
                        ps = ps_tr_p.tile([P, GW], F32, tag="tr", name=f"trb{c}")
                        for ii in range(cpg):
                            nc.tensor.transpose(
                                ps[:, ii * P:(ii + 1) * P].bitcast(tr_dt),
                                xt_in[:, ii, c * P:(c + 1) * P].bitcast(tr_dt),
                                ident)
                        # psum->sbuf copy (Pool cannot access PSUM on
                        # TRN2): alternate DVE/ACT in the prologue, where
                        # each copy gates the next projection matmul and ACT
                        # is otherwise idle; all-DVE in steady state so ACT
                        # keeps up with the exps.
                        if prologue and c % 2 == 1:
                            nc.scalar.copy(xT[:, c, :], ps[:, :W])
                        else:
                            nc.vector.tensor_copy(xT[:, c, :], ps[:, :W])

                    def emit_mm(c):
                        nc.tensor.matmul(
                            ps_vk[:, :W], wvk[:, c, :], xT[:, c, :],
                            start=(c == 0), stop=(c == NE - 1),
                            skip_group_check=True)
                        nc.tensor.matmul(
                            ps_q[:, :W], wq[:, c, :], xT[:, c, :],
                            start=(c == 0), stop=(c == NE - 1),
                            skip_group_check=True)

                    if prologue:
                        # lag the matmuls one chunk behind the transposes
                        for c in range(NE):
                            emit_trb(c)
                            if c >= 1:
                                emit_mm(c - 1)
                            yield
                        emit_mm(NE - 1)
                    else:
                        for c in range(NE):
                            emit_trb(c)
                            yield
                        for c in range(NE):
                            emit_mm(c)
                            if c % 2:
                                yield
                    # q/k/v psum->sbuf: kT/vT on DVE, qT on ACT (parallel)
                    nc.vector.tensor_copy(kT[H:, g0:g0 + W], ps_vk[H:, :W])
                    nc.scalar.copy(qT[H:, g0:g0 + W], ps_q[H:, :W])
                    nc.vector.tensor_copy(vT[:, g0:g0 + W], ps_vk[:H, :W])
                    yield
                    psv = ps_tr_p.tile([P, CPG, H], F32, tag="tr", name="psv")
                    for ii in range(cpg):
                        nc.tensor.transpose(
                            psv[:, ii, :],
                            vT[:, (jb0 + ii) * P:(jb0 + ii + 1) * P],
                            ident[:H, :H])
                    nc.vector.tensor_copy(
                        vaug[:, jb0:jb0 + cpg, :H], psv[:, :cpg])
                    yield

                atts = {}

                def attn_units(gi):
                    """scores -> exp -> AV for group gi."""
                    g0, W = GROUPS[gi]
                    jb0 = g0 // P
                    njb = jb0 + W // P           # j-blocks 0 .. njb-1
                    ps_av = ps_av_p.tile([H + 1, GW], F32, tag="av",
                                         name="ps_av")
                    pend = []                    # (j, lo, et) awaiting AV

                    def emit_av():
                        j, lo, et_j = pend.pop(0)
                        nc.tensor.matmul(
                            ps_av[:, lo:W], vaug[:, j, :], et_j[:, lo:W],
                            start=(j == 0), stop=(j == njb - 1))

                    last = gi == len(GROUPS) - 1
                    for j in range(njb):
                        rel = j - jb0
                        # diagonal blocks: t-cols below rel*P never attend
                        # this s-chunk; narrow exp/AV past them.  The score
                        # matmul keeps N >= 256 (f32r runs 4 cyc/row below
                        # that); the extra columns are real below-diagonal
                        # scores, never read downstream.
                        lo = rel * P if rel > 0 else 0
                        sc_lo = min(lo, max(0, W - 256))
                        # the last group has no tp filler: borrow the idle
                        # tr psum banks to deepen the score pipeline so PE
                        # is not gated on exp's WAR release
                        pool = (ps_tr_p if last and j % 2 else ps_sc_p)
                        ps_s = pool.tile([P, GW], F32, tag=pool is ps_tr_p
                                         and "tr" or "sc")
                        nc.tensor.matmul(
                            ps_s[:, sc_lo:W],
                            kT[H:, j * P:(j + 1) * P],
                            qT[H:, g0 + sc_lo:g0 + W],
                            start=True, stop=True)
                        et = exptp.tile([P, GW], mm_dt, tag="expt")
                        nc.scalar.activation(
                            et[:, lo:W], ps_s[:, lo:W],
                            mybir.ActivationFunctionType.Exp,
                            scale=float(H) ** -0.5)
                        if rel >= 0:
                            # causal mask applied POST-exp on the SBUF tile:
                            # zero the in-block triangle (u < s) on the Pool
                            # engine, off the score->exp critical path.
                            nc.gpsimd.affine_select(
                                out=et[:, lo:lo + P],
                                in_=et[:, lo:lo + P],
                                compare_op=mybir.AluOpType.is_ge,
                                fill=0.0, base=0,
                                pattern=[[1, P]], channel_multiplier=-1,
                            )
                        pend.append((j, lo, et))
                        yield
                        if len(pend) >= 3:
                            emit_av()
                            yield
                    while pend:
                        emit_av()
                    yield
                    atts[gi] = ps_av

                def out_units(gi):
                    """normalize + write out group gi: transpose back to
                    [t, 65], multiply rows by the reciprocal denominator
                    (row 64).  Emitted into the next group's window so the
                    avT copy hides behind its first score matmuls; the last
                    group staggers copies and stores per t-chunk to shorten
                    the serial tail."""
                    g0, W = GROUPS[gi]
                    cpg = W // P
                    jb0 = g0 // P
                    ps_av = atts.pop(gi)
                    last = gi == len(GROUPS) - 1
                    avT = outsp.tile([H + 1, GW], F32, tag="avt")
                    if last:
                        # per-chunk copies alternating DVE/ACT: chunk ii's
                        # psum columns are final after AV j-block jb0+ii
                        # (slice-granular dep), so these run while the last
                        # AV matmuls and stores still execute.
                        for ii in range(cpg):
                            reg_o = avT[:, ii * P:(ii + 1) * P]
                            reg_i = ps_av[:, ii * P:(ii + 1) * P]
                            if ii % 2 == 0:
                                nc.vector.tensor_copy(reg_o, reg_i)
                            else:
                                nc.scalar.copy(reg_o, reg_i)
                    else:
                        nc.vector.tensor_copy(avT[:, :W], ps_av[:, :W])
                    yield
                    ot = outsp.tile([P, CPG, H], F32, tag="ot")
                    for ii in range(cpg):
                        ps_o = ps_sc_p.tile([P, H + 1], F32, tag="sc",
                                            name="ps_o")
                        nc.tensor.transpose(
                            ps_o,
                            avT[:, ii * P:(ii + 1) * P],
                            ident[:H + 1, :H + 1])
                        rcp = outsp.tile([P, 1], F32, tag="rcp")
                        nc.vector.reciprocal(rcp, ps_o[:, H:H + 1])
                        nc.scalar.activation(
                            ot[:, ii, :], ps_o[:, :H],
                            mybir.ActivationFunctionType.Copy, scale=rcp)
                        if last and ii % 2 == 1:
                            # paired stores on both HWDGE queues
                            i0 = jb0 + ii - 1
                            eng = nc.sync if ii == 1 else nc.scalar
                            eng.dma_start(
                                out=out_d[i0 * P:(i0 + 2) * P, :].rearrange(
                                    "(i p) h -> p i h", p=P),
                                in_=ot[:, ii - 1:ii + 1, :])
                        yield
                    if not last:
                        nc.sync.dma_start(
                            out=out_d[g0:g0 + W, :].rearrange(
                                "(i p) h -> p i h", p=P),
                            in_=ot[:, :cpg])

                # software pipeline: window g's attention round-robins with
                # group g-1's normalize/store and a filler stream of group
                # g+1's loads/transposes/projections chained into g+1's own
                # scores/exps, so the next group's attention pre-emits
                # whenever this window has slack (this flattens the ACT exp
                # load and shrinks the final window).
                import itertools as _it
                done = object()

                def rr_until(primary, others):
                    """Round-robin primary+others until primary exhausts;
                    returns the unfinished others."""
                    gens = [primary] + [x for x in others if x is not None]
                    while True:
                        for gen in list(gens):
                            if next(gen, done) is done:
                                gens.remove(gen)
                                if gen is primary:
                                    return gens

                NGR = len(GROUPS)
                for _ in tp_units(0, loads(0), prologue=True):
                    pass
                att = attn_units(0)
                carry: list = []
                prev_out = None
                for gi in range(NGR):
                    nxt = (_it.chain(tp_units(gi + 1, loads(gi + 1)),
                                     attn_units(gi + 1))
                           if gi + 1 < NGR else None)
                    others = carry + [prev_out, nxt]
                    carry = rr_until(att, others)
                    prev_out = out_units(gi)
                    if nxt is not None:
                        if nxt in carry:
                            carry.remove(nxt)
                        att = nxt
                for gen in [prev_out] + carry:
                    for _ in gen:
                        pass

            if repeat == 1:
                body()
            else:
                tc.For_i_unrolled_general(
                    0, repeat, 1,
                    lambda iv0, unroll: body(iv0), 1,
                    hint_engines=(
                        mybir.EngineType.PE, mybir.EngineType.DVE,
                        mybir.EngineType.Activation, mybir.EngineType.SP,
                        mybir.EngineType.Pool))

    nc.compile()
    return nc


class _Runner:
    """Cached jitted SPMD executor for one built nc.

    run_bass_kernel_spmd rebuilds jax.jit(shard_map(...)) on every call,
    which forces a full XLA retrace + NEFF reload each time.  Building the
    jitted callable once (and keeping inputs device-resident) turns repeat
    calls from ~1.4 s into milliseconds, which the timing harness needs.
    """

    def __init__(self, nc):
        import jax
        from jax.experimental.shard_map import shard_map
        from jax.sharding import Mesh, NamedSharding, PartitionSpec
        from concourse import bass2jax, mybir as mb

        bass2jax.install_neuronx_cc_hook()
        in_names, out_names, out_avals = [], [], []
        for alloc in nc.m.functions[0].allocations:
            if not isinstance(alloc, mb.MemoryLocationSet):
                continue
            name = alloc.memorylocations[0].name
            if alloc.kind == "ExternalInput":
                in_names.append(name)
            elif alloc.kind == "ExternalOutput":
                out_names.append(name)
                out_avals.append(jax.core.ShapedArray(
                    tuple(alloc.tensor_shape), mb.dt.np(alloc.dtype)))
        assert nc.dbg_addr is None
        part_name = nc.partition_id_tensor.name if nc.partition_id_tensor else None
        if part_name is not None:
            in_names = [n for n in in_names if n != part_name]
        self.in_names, self.out_names, self.out_avals = in_names, out_names, out_avals
        n_params = len(in_names)
        all_names = in_names + out_names
        if part_name is not None:
            all_names = all_names + [part_name]

        def _body(*args):
            operands = list(args)
            if part_name is not None:
                operands.append(bass2jax.partition_id_tensor())
            outs = bass2jax._bass_exec_p.bind(
                *operands,
                out_avals=tuple(out_avals),
                in_names=tuple(all_names),
                out_names=tuple(out_names),
                lowering_input_output_aliases=(),
                sim_require_finite=True,
                sim_require_nnan=True,
                nc=nc,
            )
            return tuple(outs)

        devices = jax.devices()[:B]
        self.mesh = Mesh(np.asarray(devices), ("core",))
        self.spec = PartitionSpec("core")
        self.sharding = NamedSharding(self.mesh, self.spec)
        nin = n_params + len(out_names)
        self.fn = jax.jit(
            shard_map(
                _body, mesh=self.mesh,
                in_specs=(self.spec,) * nin,
                out_specs=(self.spec,) * len(out_names),
                check_rep=False,
            ),
            donate_argnums=tuple(range(n_params, nin)),
            keep_unused=True,
        )
        self._dev_inputs = {}

    def prep_inputs(self, in_maps, cache_key=None):
        """Concat per-core inputs to global arrays, optionally device-cached."""
        import jax
        if cache_key is not None and cache_key in self._dev_inputs:
            return self._dev_inputs[cache_key]
        concat = [
            np.concatenate([np.asarray(m[n]) for m in in_maps], axis=0)
            for n in self.in_names
        ]
        arrs = [jax.device_put(a, self.sharding) for a in concat]
        jax.block_until_ready(arrs)
        if cache_key is not None:
            self._dev_inputs[cache_key] = arrs
        return arrs

    def __call__(self, dev_inputs, block=True):
        import jax
        zeros = [
            np.zeros((B * av.shape[0], *av.shape[1:]), av.dtype)
            for av in self.out_avals
        ]
        outs = self.fn(*dev_inputs, *zeros)
        if block:
            jax.block_until_ready(outs)
        return outs

    def gather(self, outs):
        o = np.asarray(outs[0])
        return o.reshape(B, -1, o.shape[-1])


def _get_runner(mm_dtype: str, repeat: int) -> "_Runner":
    key = (mm_dtype, repeat)
    if key not in _NC_CACHE:
        _NC_CACHE[key] = _Runner(build_attention_nc(mm_dtype, repeat))
    return _NC_CACHE[key]


def _make_in_maps(inputs: dict):
    x = np.asarray(inputs["x"], dtype=np.float32)
    wvkq = np.ascontiguousarray(np.concatenate([
        np.asarray(inputs["Wv"], dtype=np.float32),
        np.asarray(inputs["Wk"], dtype=np.float32),
        np.asarray(inputs["Wq"], dtype=np.float32),
    ], axis=1))
    return [
        {"x": np.ascontiguousarray(x[i]), "Wvkq": wvkq}
        for i in range(B)
    ]


def run_spmd(inputs: dict, mm_dtype: str = MM_DTYPE, repeat: int = 1,
             cache_key=None):
    r = _get_runner(mm_dtype, repeat)
    dev = r.prep_inputs(_make_in_maps(inputs), cache_key=cache_key)
    return r.gather(r(dev))


def kernel(**inputs) -> np.ndarray:
    return run_spmd(inputs, MM_DTYPE, repeat=1)



# revision 61
# speedup vs baseline: 2.2024x; 2.2024x over previous
"""Single-head causal attention on 8 Trainium2 NeuronCores.

Problem: x[B=8, T=2048, E=1024] fp32, Wq/Wk/Wv [E, H=64] fp32.
    q = x @ Wq; k = x @ Wk; v = x @ Wv
    out = softmax(causal(q @ k^T / sqrt(H))) @ v          -> [8, 2048, 64]

Sharding: pure data parallel, one batch element per core; weights replicated.

Per-core kernel design (transposed-scores formulation):
  - x loads as one descriptor-efficient DMA per 512-row group ([p, i, e]
    layout, 4KB descriptors); group 0 is split into e-quarters so the
    first PE transposes start ~2us in.  Weights are host-prepacked into a
    single contiguous [Wv|Wk|Wq] tensor (one DMA, split in two halves).
  - xT[e, t] built from x via PE transpose-mode (fp32, 2 cyc/row - the
    BIR verifier requires f32r matmul operands to come from rounding
    instructions, so DMA-fed transposes cannot run at the f32r rate);
    the psum->sbuf copies (DVE, ACT in the prologue) do the f32r rounding.
  - Projections: [Wv|Wk] packed -> one M=128 matmul chain gives vT on psum
    rows 0:64 and kT on rows 64:128; [0|Wq] (zero block built on-chip)
    gives qT on rows 64:128.  kT and qT land on the SAME partitions, so
    score matmuls need no partition shift at all (operands at base 64).
  - vT -> small PE transposes -> vaug[:, j, :] = [v | 1] tiles [128, 65];
    the ones column makes the softmax denominator fall out of the AV
    matmul for free.
  - scoresT[s, t] = kT_j.T @ qT into PSUM, diagonal blocks narrowed to
    the live column range; exp(scale*x) on ACT straight from psum (no
    pre-mask); the causal triangle is zeroed POST-exp on the SBUF tile by
    a Pool-engine affine_select, off the score->exp critical path.
    exp without max-subtraction is safe: |scores| <~ 6.
  - outT[65, 512] accumulates vaug_j.T @ expT_j over j (lag-3 software
    pipeline vs exp); row 64 = softmax denominator.  Small PE transposes
    back to [t, 65], rows scaled by the reciprocal denominator via ACT
    activation(Copy, scale=rcp), one store DMA per group; the last group
    staggers per-chunk avT copies across DVE/ACT (chunk ii's psum columns
    finalize after AV j-block jb0+ii) and pairs stores on both HWDGE
    queues to shorten the serial tail.
  - Heavy matmuls run as float32r (1 cyc/row at N>=256, ~tf32-grade:
    measured rel err 2.6e-4 end to end) with an fp32 fallback
    (ATTN_MM_DTYPE=f32, 4 cycles/row).  Engine/ISA legality learned the
    hard way: GPSIMD cannot touch PSUM, f32r transpose-mode and
    tile_position column offsets fail the walrus ISA check, bf16/f32r
    operand mixes are rejected (NCC_IBIR034).
  - Software pipeline: window g round-robins attention(g) with group
    g-1's normalize/store and a filler stream of loads/transposes/
    projections(g+1) chained into attention(g+1) pre-emission; group 0's
    projection matmuls interleave per-chunk with its transposes.
"""

import os

import numpy as np

import concourse.bacc as bacc
import concourse.bass as bass
import concourse.tile as tile
from concourse import mybir
from concourse.masks import make_identity

B, T, E, H = 8, 2048, 1024, 64
P = 128                      # SBUF partitions
NE = E // P                  # 8 e-chunks
NT = T // P                  # 16 t-chunks (also s-chunks)
GW = 512                     # t-group width (matmul moving-operand max, fp32)
NG = T // GW                 # 4 t-groups
CPG = GW // P                # 4 chunks per group
F32 = mybir.dt.float32

# Matmul dtype for the heavy matmuls: "f32r" (fast) or "f32" (exact).
MM_DTYPE = os.environ.get("ATTN_MM_DTYPE", "f32r")

_NC_CACHE: dict = {}




def build_attention_nc(mm_dtype: str = "f32r", repeat: int = 1) -> bass.Bass:
    """Build the single-core Bass program (SPMD across cores via in_maps)."""
    mm_dt = {"f32": F32, "f32r": mybir.dt.float32r, "bf16": F32}[mm_dtype]
    # Transposes stay fp32 (2 cyc/row): the BIR verifier requires every
    # operand of an f32r matmul to be produced by an explicitly-rounding
    # instruction, and DMA-filled x tiles are not; a bf16 identity trips
    # NCC_IBIR034 (no 32-bit/non-32-bit mixing).  The psum->sbuf copies
    # after each transpose are the rounding step that feeds f32r matmuls.
    id_dt = F32
    tr_dt = F32

    nc = bacc.Bacc("TRN2", target_bir_lowering=False, debug=False)
    x_d = nc.dram_tensor("x", [T, E], F32, kind="ExternalInput").ap()
    # host-prepacked [Wv | Wk | 0 | Wq]: one wide contiguous weight tensor
    # loads in a single DMA; the zero block pads Wq to a full M=128
    # stationary so qT lands on psum rows 64:128 without tile_position
    # (f32r matmuls fail the walrus ISA check at column offsets).
    w_d = nc.dram_tensor("Wvkq", [E, 3 * H], F32, kind="ExternalInput").ap()
    out_d = nc.dram_tensor("out", [T, H], F32, kind="ExternalOutput").ap()

    with tile.TileContext(nc) as tc:
        with (
            tc.tile_pool(name="const", bufs=1) as const,
            tc.tile_pool(name="xin", bufs=2) as xin,
            tc.tile_pool(name="xt", bufs=2) as xtp,
            tc.tile_pool(name="proj", bufs=1) as projp,
            tc.tile_pool(name="vaug", bufs=1) as vaugp,
            tc.tile_pool(name="expt", bufs=8) as exptp,
            tc.tile_pool(name="outs", bufs=4) as outsp,
            tc.tile_pool(name="ps_sc", bufs=2, space="PSUM") as ps_sc_p,
            tc.tile_pool(name="ps_tr", bufs=2, space="PSUM") as ps_tr_p,
            tc.tile_pool(name="ps_pm", bufs=2, space="PSUM") as ps_pm_p,
            tc.tile_pool(name="ps_av", bufs=2, space="PSUM") as ps_av_p,
        ):
            # --- constants ---------------------------------------------------
            identf = const.tile([P, P], F32)
            make_identity(nc, identf)
            ident = const.tile([P, P], id_dt, tag="idc")
            nc.vector.tensor_copy(ident, identf)
            # weights, e-major: [p, c, h] with e = c*128 + p.  [Wv|Wk]
            # packed -> one M=128 matmul yields vT on psum rows 0:64 and kT
            # on rows 64:128; [0|Wq] (zero block below) pads Wq to M=128 so
            # qT also lands on rows 64:128 -> kT and qT share partitions and
            # scores need no partition shift.
            w_f = const.tile([P, NE, 3 * H], F32, tag="wf")
            wsrc = w_d.rearrange("(c p) h -> p c h", p=P)
            nc.scalar.dma_start(out=w_f[:, :NE // 2], in_=wsrc[:, :NE // 2])
            nc.scalar.dma_start(out=w_f[:, NE // 2:], in_=wsrc[:, NE // 2:])
            w_c = const.tile([P, NE, 4 * H], mm_dt, tag="wc")
            nc.gpsimd.memset(w_c[:, :, 2 * H:3 * H].bitcast(F32), 0.0)
            for c0 in range(0, NE, 2):
                pair = slice(c0, c0 + 2)
                nc.scalar.copy(w_c[:, pair, :2 * H], w_f[:, pair, :2 * H])
                nc.scalar.copy(w_c[:, pair, 3 * H:], w_f[:, pair, 2 * H:])
            wvk = w_c[:, :, :2 * H]
            wq = w_c[:, :, 2 * H:]
            ones = const.tile([P, NT, 1], F32, tag="ones")
            nc.gpsimd.memset(ones, 1.0)
            # dummy exp so the ACT table load (~1.3us) happens during the
            # DMA-bound fill, not on attention(0)'s critical path
            scr = const.tile([P, 1], F32, tag="scr")
            nc.scalar.activation(scr, ones[:, 0, :],
                                 mybir.ActivationFunctionType.Exp)

            # persistent per-iteration state (allocated fresh each repeat)
            # Variable group widths: a small first group starts attention
            # ~6us earlier during the DMA-bound fill, and a small last
            # group halves the ACT-bound final window.
            GROUPS = [(0, 512), (512, 512), (1024, 512), (1536, 512)]

            def body(_iv=None, staged=False):
                # qT/kT live on partitions 64:128 only; vT on 0:64.
                qT = projp.tile([P, T], mm_dt, tag="qt")
                kT = projp.tile([P, T], mm_dt, tag="kt")
                vT = projp.tile([H, T], F32, tag="vt")
                # vaug[s, j, :] = [v | 1] per s-chunk j; ones column via DVE
                # copy (f32r memset fails the walrus ISA check)
                vaug = vaugp.tile([P, NT, H + 1], mm_dt, tag="vaug")
                nc.vector.tensor_copy(vaug[:, :, H:H + 1], ones)

                # one-time absorber: PE picks up the Pool-engine sem for the
                # identity constant ahead of the first transposes
                dmy = ps_tr_p.tile([1, P], id_dt, tag="tr", name="dmy0")
                nc.tensor.transpose(dmy, ident[:, :1], ident)

                def loads(gi):
                    # one tile [p, i, e] holding the group rows (t = g0 +
                    # i*128 + p); group 0 is split into e-quarters so the
                    # first transposes can start ~1.5us in, later groups
                    # use halves.
                    g0, W = GROUPS[gi]
                    cpg = W // P
                    xt_in = xin.tile([P, cpg, E], F32, tag="xin", name="xin")
                    src = x_d[g0:g0 + W, :]
                    nq = 4 if gi == 0 else 2
                    for q in range(nq):
                        lo, hi = q * E // nq, (q + 1) * E // nq
                        nc.sync.dma_start(
                            out=xt_in[:, :, lo:hi],
                            in_=src[:, lo:hi].rearrange(
                                "(i p) e -> p i e", p=P))
                    return xt_in

                def tp_units(gi, xt_in, prologue=False):
                    """x-transpose + q/k/v projections + vaug build for group
                    gi (pipeline filler units).  In the prologue (group 0, no
                    attention to overlap) the projection matmuls interleave
                    per-chunk with the transposes so only the last chunk's
                    work remains after the final e-quarter lands."""
                    g0, W = GROUPS[gi]
                    cpg = W // P
                    jb0 = g0 // P
                    xT = xtp.tile([P, NE, W], mm_dt, tag="xt", name="xT")
                    ps_vk = ps_pm_p.tile([P, GW], F32, tag="pm", name="psvk")
                    ps_q = ps_pm_p.tile([P, GW], F32, tag="pm", name="psq")

                    def emit_trb(c):
                        ps = ps_tr_p.tile([P, GW], F32, tag="tr", name=f"trb{c}")
                        for ii in range(cpg):
                            nc.tensor.transpose(
                                ps[:, ii * P:(ii + 1) * P].bitcast(tr_dt),
                                xt_in[:, ii, c * P:(c + 1) * P].bitcast(tr_dt),
                                ident)
                        # psum->sbuf copy (Pool cannot access PSUM on
                        # TRN2): alternate DVE/ACT in the prologue, where
                        # each copy gates the next projection matmul and ACT
                        # is otherwise idle; all-DVE in steady state so ACT
                        # keeps up with the exps.
                        if prologue and c % 2 == 1:
                            nc.scalar.copy(xT[:, c, :], ps[:, :W])
                        else:
                            nc.vector.tensor_copy(xT[:, c, :], ps[:, :W])

                    def emit_mm(c):
                        nc.tensor.matmul(
                            ps_vk[:, :W], wvk[:, c, :], xT[:, c, :],
                            start=(c == 0), stop=(c == NE - 1),
                            skip_group_check=True)
                        nc.tensor.matmul(
                            ps_q[:, :W], wq[:, c, :], xT[:, c, :],
                            start=(c == 0), stop=(c == NE - 1),
                            skip_group_check=True)

                    if prologue:
                        # lag the matmuls one chunk behind the transposes
                        for c in range(NE):
                            emit_trb(c)
                            if c >= 1:
                                emit_mm(c - 1)
                            yield
                        emit_mm(NE - 1)
                    else:
                        for c in range(NE):
                            emit_trb(c)
                            yield
                        for c in range(NE):
                            emit_mm(c)
                            if c % 2:
                                yield
                    # q/k/v psum->sbuf: kT/vT on DVE, qT on ACT (parallel)
                    nc.vector.tensor_copy(kT[H:, g0:g0 + W], ps_vk[H:, :W])
                    nc.scalar.copy(qT[H:, g0:g0 + W], ps_q[H:, :W])
                    nc.vector.tensor_copy(vT[:, g0:g0 + W], ps_vk[:H, :W])
                    yield
                    psv = ps_tr_p.tile([P, CPG, H], F32, tag="tr", name="psv")
                    for ii in range(cpg):
                        nc.tensor.transpose(
                            psv[:, ii, :],
                            vT[:, (jb0 + ii) * P:(jb0 + ii + 1) * P],
                            ident[:H, :H])
                    nc.vector.tensor_copy(
                        vaug[:, jb0:jb0 + cpg, :H], psv[:, :cpg])
                    yield

                atts = {}

                def attn_units(gi):
                    """scores -> exp -> AV for group gi."""
                    g0, W = GROUPS[gi]
                    jb0 = g0 // P
                    njb = jb0 + W // P           # j-blocks 0 .. njb-1
                    ps_av = ps_av_p.tile([H + 1, GW], F32, tag="av",
                                         name="ps_av")
                    pend = []                    # (j, lo, et) awaiting AV

                    def emit_av():
                        j, lo, et_j = pend.pop(0)
                        nc.tensor.matmul(
                            ps_av[:, lo:W], vaug[:, j, :], et_j[:, lo:W],
                            start=(j == 0), stop=(j == njb - 1))

                    last = gi == len(GROUPS) - 1
                    for j in range(njb):
                        rel = j - jb0
                        # diagonal blocks: t-cols below rel*P never attend
                        # this s-chunk; narrow exp/AV past them.  The score
                        # matmul keeps N >= 256 (f32r runs 4 cyc/row below
                        # that); the extra columns are real below-diagonal
                        # scores, never read downstream.
                        lo = rel * P if rel > 0 else 0
                        sc_lo = min(lo, max(0, W - 256))
                        # the last group has no tp filler: borrow the idle
                        # tr psum banks to deepen the score pipeline so PE
                        # is not gated on exp's WAR release
                        pool = (ps_tr_p if last and j % 2 else ps_sc_p)
                        ps_s = pool.tile([P, GW], F32, tag=pool is ps_tr_p
                                         and "tr" or "sc")
                        nc.tensor.matmul(
                            ps_s[:, sc_lo:W],
                            kT[H:, j * P:(j + 1) * P],
                            qT[H:, g0 + sc_lo:g0 + W],
                            start=True, stop=True)
                        et = exptp.tile([P, GW], mm_dt, tag="expt")
                        nc.scalar.activation(
                            et[:, lo:W], ps_s[:, lo:W],
                            mybir.ActivationFunctionType.Exp,
                            scale=float(H) ** -0.5)
                        if rel >= 0:
                            # causal mask applied POST-exp on the SBUF tile:
                            # zero the in-block triangle (u < s) on the Pool
                            # engine, off the score->exp critical path.
                            nc.gpsimd.affine_select(
                                out=et[:, lo:lo + P],
                                in_=et[:, lo:lo + P],
                                compare_op=mybir.AluOpType.is_ge,
                                fill=0.0, base=0,
                                pattern=[[1, P]], channel_multiplier=-1,
                            )
                        pend.append((j, lo, et))
                        yield
                        if len(pend) >= 3:
                            emit_av()
                            yield
                    while pend:
                        emit_av()
                    yield
                    atts[gi] = ps_av

                def out_units(gi):
                    """normalize + write out group gi: transpose back to
                    [t, 65], multiply rows by the reciprocal denominator
                    (row 64).  Emitted into the next group's window so the
                    avT copy hides behind its first score matmuls; the last
                    group staggers copies and stores per t-chunk to shorten
                    the serial tail."""
                    g0, W = GROUPS[gi]
                    cpg = W // P
                    jb0 = g0 // P
                    ps_av = atts.pop(gi)
                    last = gi == len(GROUPS) - 1
                    avT = outsp.tile([H + 1, GW], F32, tag="avt")
                    if last:
                        # per-chunk copies alternating DVE/ACT: chunk ii's
                        # psum columns are final after AV j-block jb0+ii
                        # (slice-granular dep), so these run while the last
                        # AV matmuls and stores still execute.
                        for ii in range(cpg):
                            reg_o = avT[:, ii * P:(ii + 1) * P]
                            reg_i = ps_av[:, ii * P:(ii + 1) * P]
                            if ii % 2 == 0:
                                nc.vector.tensor_copy(reg_o, reg_i)
                            else:
                                nc.scalar.copy(reg_o, reg_i)
                    else:
                        nc.vector.tensor_copy(avT[:, :W], ps_av[:, :W])
                    yield
                    ot = outsp.tile([P, CPG, H], F32, tag="ot")
                    for ii in range(cpg):
                        ps_o = ps_sc_p.tile([P, H + 1], F32, tag="sc",
                                            name="ps_o")
                        nc.tensor.transpose(
                            ps_o,
                            avT[:, ii * P:(ii + 1) * P],
                            ident[:H + 1, :H + 1])
                        rcp = outsp.tile([P, 1], F32, tag="rcp")
                        nc.vector.reciprocal(rcp, ps_o[:, H:H + 1])
                        nc.scalar.activation(
                            ot[:, ii, :], ps_o[:, :H],
                            mybir.ActivationFunctionType.Copy, scale=rcp)
                        if last and ii % 2 == 1:
                            # paired stores on both HWDGE queues
                            i0 = jb0 + ii - 1
                            eng = nc.sync if ii == 1 else nc.scalar
                            eng.dma_start(
                                out=out_d[i0 * P:(i0 + 2) * P, :].rearrange(
                                    "(i p) h -> p i h", p=P),
                                in_=ot[:, ii - 1:ii + 1, :])
                        yield
                    if not last:
                        nc.sync.dma_start(
                            out=out_d[g0:g0 + W, :].rearrange(
                                "(i p) h -> p i h", p=P),
                            in_=ot[:, :cpg])

                # software pipeline: window g's attention round-robins with
                # group g-1's normalize/store and a filler stream of group
                # g+1's loads/transposes/projections chained into g+1's own
                # scores/exps, so the next group's attention pre-emits
                # whenever this window has slack (this flattens the ACT exp
                # load and shrinks the final window).
                import itertools as _it
                done = object()

                def rr_until(primary, others):
                    """Round-robin primary+others until primary exhausts;
                    returns the unfinished others."""
                    gens = [primary] + [x for x in others if x is not None]
                    while True:
                        for gen in list(gens):
                            if next(gen, done) is done:
                                gens.remove(gen)
                                if gen is primary:
                                    return gens

                NGR = len(GROUPS)
                for _ in tp_units(0, loads(0), prologue=True):
                    pass
                att = attn_units(0)
                carry: list = []
                prev_out = None
                for gi in range(NGR):
                    nxt = (_it.chain(tp_units(gi + 1, loads(gi + 1)),
                                     attn_units(gi + 1))
                           if gi + 1 < NGR else None)
                    others = carry + [prev_out, nxt]
                    carry = rr_until(att, others)
                    prev_out = out_units(gi)
                    if nxt is not None:
                        if nxt in carry:
                            carry.remove(nxt)
                        att = nxt
                for gen in [prev_out] + carry:
                    for _ in gen:
                        pass

            if repeat == 1:
                body()
            else:
                tc.For_i_unrolled_general(
                    0, repeat, 1,
                    lambda iv0, unroll: body(iv0), 1,
                    hint_engines=(
                        mybir.EngineType.PE, mybir.EngineType.DVE,
                        mybir.EngineType.Activation, mybir.EngineType.SP,
                        mybir.EngineType.Pool))

    nc.compile()
    return nc


class _Runner:
    """Cached jitted SPMD executor for one built nc.

    run_bass_kernel_spmd rebuilds jax.jit(shard_map(...)) on every call,
    which forces a full XLA retrace + NEFF reload each time.  Building the
    jitted callable once (and keeping inputs device-resident) turns repeat
    calls from ~1.4 s into milliseconds, which the timing harness needs.
    """

    def __init__(self, nc):
        import jax
        from jax.experimental.shard_map import shard_map
        from jax.sharding import Mesh, NamedSharding, PartitionSpec
        from concourse import bass2jax, mybir as mb

        bass2jax.install_neuronx_cc_hook()
        in_names, out_names, out_avals = [], [], []
        for alloc in nc.m.functions[0].allocations:
            if not isinstance(alloc, mb.MemoryLocationSet):
                continue
            name = alloc.memorylocations[0].name
            if alloc.kind == "ExternalInput":
                in_names.append(name)
            elif alloc.kind == "ExternalOutput":
                out_names.append(name)
                out_avals.append(jax.core.ShapedArray(
                    tuple(alloc.tensor_shape), mb.dt.np(alloc.dtype)))
        assert nc.dbg_addr is None
        part_name = nc.partition_id_tensor.name if nc.partition_id_tensor else None
        if part_name is not None:
            in_names = [n for n in in_names if n != part_name]
        self.in_names, self.out_names, self.out_avals = in_names, out_names, out_avals
        n_params = len(in_names)
        all_names = in_names + out_names
        if part_name is not None:
            all_names = all_names + [part_name]

        def _body(*args):
            operands = list(args)
            if part_name is not None:
                operands.append(bass2jax.partition_id_tensor())
            outs = bass2jax._bass_exec_p.bind(
                *operands,
                out_avals=tuple(out_avals),
                in_names=tuple(all_names),
                out_names=tuple(out_names),
                lowering_input_output_aliases=(),
                sim_require_finite=True,
                sim_require_nnan=True,
                nc=nc,
            )
            return tuple(outs)

        devices = jax.devices()[:B]
        self.mesh = Mesh(np.asarray(devices), ("core",))
        self.spec = PartitionSpec("core")
        self.sharding = NamedSharding(self.mesh, self.spec)
        nin = n_params + len(out_names)
        self.fn = jax.jit(
            shard_map(
                _body, mesh=self.mesh,
                in_specs=(self.spec,) * nin,
                out_specs=(self.spec,) * len(out_names),
                check_rep=False,
            ),
            donate_argnums=tuple(range(n_params, nin)),
            keep_unused=True,
        )
        self._dev_inputs = {}

    def prep_inputs(self, in_maps, cache_key=None):
        """Concat per-core inputs to global arrays, optionally device-cached."""
        import jax
        if cache_key is not None and cache_key in self._dev_inputs:
            return self._dev_inputs[cache_key]
        concat = [
            np.concatenate([np.asarray(m[n]) for m in in_maps], axis=0)
            for n in self.in_names
        ]
        arrs = [jax.device_put(a, self.sharding) for a in concat]
        jax.block_until_ready(arrs)
        if cache_key is not None:
            self._dev_inputs[cache_key] = arrs
        return arrs

    def __call__(self, dev_inputs, block=True):
        import jax
        zeros = [
            np.zeros((B * av.shape[0], *av.shape[1:]), av.dtype)
            for av in self.out_avals
        ]
        outs = self.fn(*dev_inputs, *zeros)
        if block:
            jax.block_until_ready(outs)
        return outs

    def gather(self, outs):
        o = np.asarray(outs[0])
        return o.reshape(B, -1, o.shape[-1])


def _get_runner(mm_dtype: str, repeat: int) -> "_Runner":
    key = (mm_dtype, repeat)
    if key not in _NC_CACHE:
        _NC_CACHE[key] = _Runner(build_attention_nc(mm_dtype, repeat))
    return _NC_CACHE[key]


def _make_in_maps(inputs: dict):
    x = np.asarray(inputs["x"], dtype=np.float32)
    wvkq = np.ascontiguousarray(np.concatenate([
        np.asarray(inputs["Wv"], dtype=np.float32),
        np.asarray(inputs["Wk"], dtype=np.float32),
        np.asarray(inputs["Wq"], dtype=np.float32),
    ], axis=1))
    return [
        {"x": np.ascontiguousarray(x[i]), "Wvkq": wvkq}
        for i in range(B)
    ]


def run_spmd(inputs: dict, mm_dtype: str = MM_DTYPE, repeat: int = 1,
             cache_key=None):
    r = _get_runner(mm_dtype, repeat)
    dev = r.prep_inputs(_make_in_maps(inputs), cache_key=cache_key)
    return r.gather(r(dev))


def kernel(**inputs) -> np.ndarray:
    return run_spmd(inputs, MM_DTYPE, repeat=1)

